# revision 8
# baseline (speedup 1.0000x reference)
"""Trainium2 Bass kernel for nn_NeuralMemory (Titans-style chunked neural memory).

Strategy (8 NeuronCores, SPMD single program, shard identity carried by data):
  * Tensor-parallel over the memory MLP hidden dim (1024 -> 128 per core).
  * Prep: each core computes k/k^T/v/v^T for 2 of the 16 chunks; one AllGather
    shares them (replicated in HBM).
  * Scan (16 sequential chunk steps): forward h_s/a_s and pred-partial are
    hid-sharded; pred partials are AllReduced in bf16 (1 collective/step).
    Backward (g1_s, g2_s) and weight updates are local to each shard.
  * Final pass: tokens sharded 8-way; full W1/W2 assembled via one AllGather.
All matmuls run as float32r (tf32-like) with fp32 PSUM accumulation.
"""

import sys

sys.path.insert(0, "/opt/trn_rl_repo")

import numpy as np

import concourse.bass as bass  # noqa: F401
import concourse.tile as tile
from concourse import bacc, mybir
from concourse.bass_utils import run_bass_kernel_spmd

F32 = mybir.dt.float32
F32R = mybir.dt.float32r
BF16 = mybir.dt.bfloat16
AF = mybir.ActivationFunctionType
ALU = mybir.AluOpType

B, S, D = 4, 4096, 512
CH = 256
NCH = 16
TOK = B * CH  # 1024 tokens per chunk
H = 2 * D  # 1024
NCORES = 8
HS = H // NCORES  # 128 hid per core
CPC = NCH // NCORES  # chunks per core in prep = 2
TOKQ = (B * S) // NCORES  # 2048 final tokens per core
C2N = 2.0 / float(B * CH * D)
CHUNK_F = 2 * 512 * 1024  # floats per chunk in f32 AG buffer (kT + k)
CHUNK_H = 2 * 512 * 1024  # elems per chunk in bf16 AG buffer (v + vT)


def _mm_acc(nc, ps, lhsT_fn, rhs_fn, nk):
    for kk in range(nk):
        nc.tensor.matmul(ps, lhsT_fn(kk), rhs_fn(kk), start=(kk == 0), stop=(kk == nk - 1))


def build_program(update_mem, alpha, lr, decay, n_chunks=NCH):
    nc = bacc.Bacc("TRN2", target_bir_lowering=False, debug=False, num_devices=NCORES)

    # ---------------- I/O ----------------
    xTp_in = nc.dram_tensor("xTp", [CPC, 4, 128, B * CH], F32R, kind="ExternalInput")
    xTq_in = nc.dram_tensor("xTq", [4, 128, TOKQ], F32R, kind="ExternalInput")
    wq_in = nc.dram_tensor("wq", [4, 128, 512], F32R, kind="ExternalInput")
    wk_in = nc.dram_tensor("wk", [4, 128, 512], F32R, kind="ExternalInput")
    wv_in = nc.dram_tensor("wv", [4, 128, 512], F32R, kind="ExternalInput")
    w1s_in = nc.dram_tensor("w1s", [4, 128, HS], F32, kind="ExternalInput")
    w2s_in = nc.dram_tensor("w2s", [HS, 512], F32, kind="ExternalInput")
    w2ts_in = nc.dram_tensor("w2ts", [4, 128, HS], F32, kind="ExternalInput")
    ident_in = nc.dram_tensor("ident", [128, 128], F32, kind="ExternalInput")
    identr_in = nc.dram_tensor("identr", [128, 128], F32R, kind="ExternalInput")
    y_out = nc.dram_tensor("y", [TOKQ, 512], F32, kind="ExternalOutput")

    glr = -lr * C2N  # gradient copy-out scale (folds loss normalization)

    with tile.TileContext(nc) as tc:
        from contextlib import ExitStack

        with ExitStack() as ctx:
            const = ctx.enter_context(tc.tile_pool(name="const", bufs=1))
            psum = ctx.enter_context(tc.tile_pool(name="psum", bufs=4, space="PSUM"))
            dram = ctx.enter_context(tc.tile_pool(name="dram", bufs=1, space="DRAM"))
            dram2 = ctx.enter_context(tc.tile_pool(name="dram2", bufs=2, space="DRAM"))

            ident = const.tile([128, 128], F32, tag="ident")
            nc.sync.dma_start(ident[:], ident_in[:])
            identr_t = const.tile([128, 128], F32R, tag="identr")
            nc.sync.dma_start(identr_t[:], identr_in.ap())
            identr = identr_t[:]

            # persistent weight state (masters f32, matmul copies f32r)
            W1m = const.tile([128, 4, HS], F32, tag="W1m")
            W2m = const.tile([128, 512], F32, tag="W2m")
            W2Tm = const.tile([128, 4, HS], F32, tag="W2Tm")
            nc.sync.dma_start(W1m[:], w1s_in.ap().rearrange("k p m -> p k m"))
            nc.sync.dma_start(W2m[:], w2s_in[:])
            nc.sync.dma_start(W2Tm[:], w2ts_in.ap().rearrange("k p m -> p k m"))
            W1r = const.tile([128, 4, HS], F32R, tag="W1r")
            W2r = const.tile([128, 512], F32R, tag="W2r")
            W2Tr = const.tile([128, 4, HS], F32R, tag="W2Tr")
            nc.vector.tensor_copy(W1r[:], W1m[:])
            nc.vector.tensor_copy(W2r[:], W2m[:])
            nc.vector.tensor_copy(W2Tr[:], W2Tm[:])
            S1 = const.tile([128, 4, HS], F32, tag="S1")
            S2 = const.tile([128, 512], F32, tag="S2")
            S2T = const.tile([128, 4, HS], F32, tag="S2T")
            nc.vector.memset(S1[:], 0.0)
            nc.vector.memset(S2[:], 0.0)
            nc.vector.memset(S2T[:], 0.0)

            # DRAM scratch
            qTd = dram.tile([4, 128, TOKQ], F32R, tag="qTd")
            agin_f = dram.tile([CPC, CHUNK_F], F32, tag="agin_f")
            agout_f = dram.tile([NCORES, CPC, CHUNK_F], F32, tag="agout_f", addr_space="Shared")
            agin_h = dram.tile([CPC, CHUNK_H], BF16, tag="agin_h")
            agout_h = dram.tile([NCORES, CPC, CHUNK_H], BF16, tag="agout_h", addr_space="Shared")
            wag_in = dram.tile([4 * 128 * HS + HS * 512], F32, tag="wag_in")
            wag_out = dram.tile([NCORES, 4 * 128 * HS + HS * 512], F32, tag="wag_out", addr_space="Shared")

            # ---------------- prep phase ----------------
            with tc.tile_pool(name="prep", bufs=2) as prep:
                wq = prep.tile([128, 4, 512], F32R, tag="wq")
                wk = prep.tile([128, 4, 512], F32R, tag="wk")
                wv = prep.tile([128, 4, 512], F32R, tag="wv")
                nc.sync.dma_start(wq[:], wq_in.ap().rearrange("k p n -> p k n"))
                nc.sync.dma_start(wk[:], wk_in.ap().rearrange("k p n -> p k n"))
                nc.sync.dma_start(wv[:], wv_in.ap().rearrange("k p n -> p k n"))

                # qT for the final pass (token shard of this core)
                xTq = prep.tile([128, 4, TOKQ], F32R, tag="xTq")
                nc.sync.dma_start(xTq[:], xTq_in.ap().rearrange("k p n -> p k n"))
                for m in range(4):
                    for ns in range(TOKQ // 512):
                        ps = psum.tile([128, 512], F32, tag="mm")
                        _mm_acc(
                            nc, ps[:],
                            lambda kk, m=m: wq[:, kk, m * 128:(m + 1) * 128],
                            lambda kk, ns=ns: xTq[:, kk, ns * 512:(ns + 1) * 512],
                            4,
                        )
                        qs = prep.tile([128, 512], F32R, tag="qstage")
                        nc.vector.tensor_copy(qs[:], ps[:])
                        nc.sync.dma_start(
                            qTd[:, :, ns * 512:(ns + 1) * 512]
                            .rearrange("k p n -> p k n")[:, m, :],
                            qs[:],
                        )

                if update_mem:
                    for i in range(CPC):
                        XTp = prep.tile([128, 4, B * CH], F32R, tag="XTp")
                        nc.sync.dma_start(
                            XTp[:],
                            xTp_in.ap()[i].rearrange("k p n -> p k n"),
                        )
                        XTf = XTp

                        # kT chunk: [512(d2), 1024(tok)] stored [m p n]
                        for m in range(4):
                            for nh in range(2):
                                ps = psum.tile([128, 512], F32, tag="mm")
                                _mm_acc(
                                    nc, ps[:],
                                    lambda kk, m=m: wk[:, kk, m * 128:(m + 1) * 128],
                                    lambda kk, nh=nh: XTf[:, kk, nh * 512:(nh + 1) * 512],
                                    4,
                                )
                                st = prep.tile([128, 512], F32, tag="pstage")
                                nc.scalar.copy(st[:], ps[:])
                                nc.sync.dma_start(
                                    agin_f[i, 0:512 * 1024]
                                    .rearrange("(m p n) -> p m n", m=4, p=128, n=1024)
                                    [:, m, nh * 512:(nh + 1) * 512],
                                    st[:],
                                )
                        # k chunk: [1024(tok), 512] stored [mt p n]
                        for mt in range(8):
                            ps = psum.tile([128, 512], F32, tag="mm")
                            _mm_acc(
                                nc, ps[:],
                                lambda kk, mt=mt: XTp[:, kk, mt * 128:(mt + 1) * 128],
                                lambda kk: wk[:, kk, :],
                                4,
                            )
                            st = prep.tile([128, 512], F32, tag="pstage")
                            nc.scalar.copy(st[:], ps[:])
                            nc.sync.dma_start(
                                agin_f[i, 512 * 1024:2 * 512 * 1024]
                                .rearrange("(mt p n) -> p mt n", mt=8, p=128, n=512)[:, mt, :],
                                st[:],
                            )
                        # v chunk (bf16): [1024(tok), 512] + vT [512(d2), 1024]
                        for mt in range(8):
                            ps = psum.tile([128, 512], F32, tag="mm")
                            _mm_acc(
                                nc, ps[:],
                                lambda kk, mt=mt: XTp[:, kk, mt * 128:(mt + 1) * 128],
                                lambda kk: wv[:, kk, :],
                                4,
                            )
                            sth = prep.tile([128, 512], BF16, tag="hstage")
                            nc.vector.tensor_copy(sth[:], ps[:])
                            nc.sync.dma_start(
                                agin_h[i, 0:512 * 1024]
                                .rearrange("(mt p n) -> p mt n", mt=8, p=128, n=512)[:, mt, :],
                                sth[:],
                            )
                        for m in range(4):
                            for nh in range(2):
                                ps = psum.tile([128, 512], F32, tag="mm")
                                _mm_acc(
                                    nc, ps[:],
                                    lambda kk, m=m: wv[:, kk, m * 128:(m + 1) * 128],
                                    lambda kk, nh=nh: XTf[:, kk, nh * 512:(nh + 1) * 512],
                                    4,
                                )
                                sth = prep.tile([128, 512], BF16, tag="hstage")
                                nc.vector.tensor_copy(sth[:], ps[:])
                                nc.sync.dma_start(
                                    agin_h[i, 512 * 1024:2 * 512 * 1024]
                                    .rearrange("(m p n) -> p m n", m=4, p=128, n=1024)
                                    [:, m, nh * 512:(nh + 1) * 512],
                                    sth[:],
                                )

            if update_mem:
                nc.gpsimd.collective_compute(
                    "AllGather",
                    ALU.bypass,
                    replica_groups=[list(range(NCORES))],
                    ins=[agin_f[:].opt()],
                    outs=[agout_f[:].opt()],
                )
                nc.gpsimd.collective_compute(
                    "AllGather",
                    ALU.bypass,
                    replica_groups=[list(range(NCORES))],
                    ins=[agin_h[:].opt()],
                    outs=[agout_h[:].opt()],
                )

            # ---------------- scan ----------------
            if update_mem:
                with tc.tile_pool(name="scan", bufs=1) as sc:
                    for t in range(n_chunks):
                        r, i = t // CPC, t % CPC
                        kT_t = sc.tile([128, 4, 1024], F32R, tag="kT_t", bufs=1)
                        nc.sync.dma_start(
                            kT_t[:],
                            agout_f[r, i, 0:512 * 1024]
                            .rearrange("(m p n) -> p m n", m=4, p=128, n=1024)
                            .bitcast(F32R),
                        )
                        k_t = sc.tile([128, 8, 512], F32R, tag="k_t", bufs=2)
                        nc.sync.dma_start(
                            k_t[:],
                            agout_f[r, i, 512 * 1024:2 * 512 * 1024]
                            .rearrange("(mt p n) -> p mt n", mt=8, p=128, n=512)
                            .bitcast(F32R),
                        )
                        v_t = sc.tile([128, 8, 512], BF16, tag="v_t", bufs=1)
                        nc.sync.dma_start(
                            v_t[:],
                            agout_h[r, i, 0:512 * 1024]
                            .rearrange("(mt p n) -> p mt n", mt=8, p=128, n=512),
                        )
                        vT_t = sc.tile([128, 4, 1024], BF16, tag="vT_t", bufs=1)
                        nc.sync.dma_start(
                            vT_t[:],
                            agout_h[r, i, 512 * 1024:2 * 512 * 1024]
                            .rearrange("(m p n) -> p m n", m=4, p=128, n=1024),
                        )

                        # forward: h_s^T, a_s^T  [128(hid_s), 1024(tok)]
                        h_sT = sc.tile([128, 1024], F32, tag="h_sT", bufs=2)
                        a_sT = sc.tile([128, 1024], F32R, tag="a_sT", bufs=2)
                        for nh in range(2):
                            ps = psum.tile([128, 512], F32, tag="mm")
                            _mm_acc(
                                nc, ps[:],
                                lambda kk: W1r[:, kk, :],
                                lambda kk, nh=nh: kT_t[:, kk, nh * 512:(nh + 1) * 512],
                                4,
                            )
                            nc.scalar.copy(h_sT[:, nh * 512:(nh + 1) * 512], ps[:])
                            nc.scalar.activation(
                                a_sT[:, nh * 512:(nh + 1) * 512], ps[:], AF.Gelu
                            )

                        # a_s token layout via PE transpose
                        a_s = sc.tile([128, 8, 128], F32R, tag="a_s", bufs=2)
                        for j in range(8):
                            tp = psum.tile([128, 128], F32, tag="tp")
                            nc.tensor.transpose(tp[:].bitcast(F32R), a_sT[:, j * 128:(j + 1) * 128], identr)
                            nc.vector.tensor_copy(a_s[:, j, :], tp[:])

                        # pred partial (token layout) -> bf16 -> AllReduce
                        arin = dram2.tile([TOK * 512], BF16, tag="arin")
                        for mt in range(8):
                            ps = psum.tile([128, 512], F32, tag="mm")
                            nc.tensor.matmul(
                                ps[:], a_sT[:, mt * 128:(mt + 1) * 128], W2r[:],
                                start=True, stop=True,
                            )
                            pb = sc.tile([128, 512], BF16, tag="partb", bufs=2)
                            nc.vector.tensor_copy(pb[:], ps[:])
                            nc.sync.dma_start(
                                arin[mt * 65536:(mt + 1) * 65536]
                                .rearrange("(p n) -> p n", p=128, n=512),
                                pb[:],
                            )
                        arout = dram2.tile([TOK * 512], BF16, tag="arout", addr_space="Shared")
                        nc.gpsimd.collective_compute(
                            "AllReduce",
                            ALU.add,
                            replica_groups=[list(range(NCORES))],
                            ins=[arin[:].opt()],
                            outs=[arout[:].opt()],
                        )

                        pred = sc.tile([128, 8, 512], BF16, tag="pred")
                        nc.sync.dma_start(
                            pred[:],
                            arout[:].rearrange("(mt p n) -> p mt n", mt=8, p=128, n=512),
                        )
                        predT = sc.tile([128, 4, 1024], BF16, tag="predT")
                        ar2d = arout[:].rearrange("(t n) -> t n", t=1024, n=512)
                        for kk in range(4):
                            nc.sync.dma_start_transpose(
                                predT[:, kk, :], ar2d[:, kk * 128:(kk + 1) * 128]
                            )

                        # d_pred (token, f32r) and d_pred^T (feature, f32r); raw (pred - v)
                        d_pred = sc.tile([128, 8, 512], F32R, tag="d_pred")
                        nc.vector.tensor_sub(
                            d_pred[:].rearrange("p a b -> p (a b)"),
                            pred[:].rearrange("p a b -> p (a b)"),
                            v_t[:].rearrange("p a b -> p (a b)"),
                        )
                        d_predT = sc.tile([128, 4, 1024], F32R, tag="d_predT")
                        nc.vector.tensor_sub(
                            d_predT[:].rearrange("p a b -> p (a b)"),
                            predT[:].rearrange("p a b -> p (a b)"),
                            vT_t[:].rearrange("p a b -> p (a b)"),
                        )

                        # g2_s = a_s^T d_pred (hid_s x 512); scaled by -lr*c2N on copy-out
                        g2l = sc.tile([128, 512], F32, tag="g2l")
                        ps = psum.tile([128, 512], F32, tag="mm")
                        _mm_acc(
                            nc, ps[:],
                            lambda kk: a_s[:, kk, :],
                            lambda kk: d_pred[:, kk, :],
                            8,
                        )
                        nc.scalar.mul(g2l[:], ps[:], glr)
                        # g2T_s (512 x hid_s) stored [128,4,128]
                        g2Tl = sc.tile([128, 4, 128], F32, tag="g2Tl")
                        for m in range(4):
                            tp = psum.tile([128, 128], F32, tag="tp")
                            _mm_acc(
                                nc, tp[:],
                                lambda kk, m=m: d_pred[:, kk, m * 128:(m + 1) * 128],
                                lambda kk: a_s[:, kk, :],
                                8,
                            )
                            nc.scalar.mul(g2Tl[:, m, :], tp[:], glr)

                        # d_a^T = W2_s @ d_pred^T ; d_h^T = d_a^T * gelu'(h^T)
                        d_hT = sc.tile([128, 1024], F32R, tag="d_hT")
                        for nh in range(2):
                            ps = psum.tile([128, 512], F32, tag="mm")
                            _mm_acc(
                                nc, ps[:],
                                lambda kk: W2Tr[:, kk, :],
                                lambda kk, nh=nh: d_predT[:, kk, nh * 512:(nh + 1) * 512],
                                4,
                            )
                            dg = sc.tile([128, 512], F32, tag="dg", bufs=2)
                            nc.scalar.activation(
                                dg[:], h_sT[:, nh * 512:(nh + 1) * 512], AF.Derivative_Gelu
                            )
                            nc.vector.tensor_mul(
                                d_hT[:, nh * 512:(nh + 1) * 512], dg[:], ps[:]
                            )

                        # d_h token layout
                        d_h = sc.tile([128, 8, 128], F32R, tag="d_h")
                        for j in range(8):
                            tp = psum.tile([128, 128], F32, tag="tp")
                            nc.tensor.transpose(tp[:].bitcast(F32R), d_hT[:, j * 128:(j + 1) * 128], identr)
                            nc.vector.tensor_copy(d_h[:, j, :], tp[:])

                        # g1T_s [hid_s, 512] then transpose to g1l [512, hid_s]
                        g1Tl = sc.tile([128, 512], F32, tag="g1Tl")
                        ps = psum.tile([128, 512], F32, tag="mm")
                        _mm_acc(
                            nc, ps[:],
                            lambda kk: d_h[:, kk, :],
                            lambda kk: k_t[:, kk, :],
                            8,
                        )
                        nc.scalar.mul(g1Tl[:], ps[:], glr)
                        g1l = sc.tile([128, 4, 128], F32, tag="g1l")
                        for m in range(4):
                            tp = psum.tile([128, 128], F32, tag="tp")
                            nc.tensor.transpose(tp[:], g1Tl[:, m * 128:(m + 1) * 128], ident[:])
                            nc.vector.tensor_copy(g1l[:, m, :], tp[:])

                        # updates: S = decay*S + gl ; W = (1-alpha)*W + S ; refresh f32r copies
                        nc.vector.scalar_tensor_tensor(
                            S1[:].rearrange("p a b -> p (a b)"), S1[:].rearrange("p a b -> p (a b)"),
                            decay, g1l[:].rearrange("p a b -> p (a b)"), ALU.mult, ALU.add,
                        )
                        nc.vector.scalar_tensor_tensor(
                            W1m[:].rearrange("p a b -> p (a b)"), W1m[:].rearrange("p a b -> p (a b)"),
                            1.0 - alpha, S1[:].rearrange("p a b -> p (a b)"), ALU.mult, ALU.add,
                        )
                        nc.vector.tensor_copy(W1r[:], W1m[:])
                        nc.vector.scalar_tensor_tensor(
                            S2[:], S2[:], decay, g2l[:], ALU.mult, ALU.add
                        )
                        nc.vector.scalar_tensor_tensor(
                            W2m[:], W2m[:], 1.0 - alpha, S2[:], ALU.mult, ALU.add
                        )
                        nc.vector.tensor_copy(W2r[:], W2m[:])
                        nc.vector.scalar_tensor_tensor(
                            S2T[:].rearrange("p a b -> p (a b)"), S2T[:].rearrange("p a b -> p (a b)"),
                            decay, g2Tl[:].rearrange("p a b -> p (a b)"), ALU.mult, ALU.add,
                        )
                        nc.vector.scalar_tensor_tensor(
                            W2Tm[:].rearrange("p a b -> p (a b)"), W2Tm[:].rearrange("p a b -> p (a b)"),
                            1.0 - alpha, S2T[:].rearrange("p a b -> p (a b)"), ALU.mult, ALU.add,
                        )
                        nc.vector.tensor_copy(W2Tr[:], W2Tm[:])

            # ---------------- gather full W, final pass ----------------
            nc.sync.dma_start(
                wag_in[0:4 * 128 * HS].rearrange("(k p m) -> p k m", k=4, p=128, m=HS),
                W1m[:],
            )
            nc.sync.dma_start(
                wag_in[4 * 128 * HS:].rearrange("(p n) -> p n", p=HS, n=512),
                W2m[:],
            )
            nc.gpsimd.collective_compute(
                "AllGather",
                ALU.bypass,
                replica_groups=[list(range(NCORES))],
                ins=[wag_in[:].opt()],
                outs=[wag_out[:].opt()],
            )

            with tc.tile_pool(name="fin", bufs=2) as fin:
                W1f = fin.tile([128, 4, 8, 128], F32R, tag="W1f", bufs=1)
                for kk in range(4):
                    nc.sync.dma_start(
                        W1f[:, kk, :, :],
                        wag_out[:, kk * 128 * HS:(kk + 1) * 128 * HS]
                        .rearrange("r (p m) -> p r m", p=128, m=HS)
                        .bitcast(F32R),
                    )
                W2f = fin.tile([128, 8, 512], F32R, tag="W2f", bufs=1)
                nc.sync.dma_start(
                    W2f[:],
                    wag_out[:, 4 * 128 * HS:]
                    .rearrange("r (p n) -> p r n", p=HS, n=512)
                    .bitcast(F32R),
                )

                for ts in range(TOKQ // 512):
                    qT_sl = fin.tile([128, 4, 512], F32R, tag="qT_sl")
                    nc.sync.dma_start(
                        qT_sl[:],
                        qTd[:, :, ts * 512:(ts + 1) * 512].rearrange("k p n -> p k n"),
                    )
                    aT_sl = fin.tile([128, 8, 512], F32R, tag="aT_sl")
                    for rr in range(8):
                        ps = psum.tile([128, 512], F32, tag="mm")
                        _mm_acc(
                            nc, ps[:],
                            lambda kk, rr=rr: W1f[:, kk, rr, :],
                            lambda kk: qT_sl[:, kk, :],
                            4,
                        )
                        nc.scalar.activation(aT_sl[:, rr, :], ps[:], AF.Gelu)
                    for mt in range(4):
                        ps = psum.tile([128, 512], F32, tag="mm")
                        _mm_acc(
                            nc, ps[:],
                            lambda kk, mt=mt: aT_sl[:, kk, mt * 128:(mt + 1) * 128],
                            lambda kk: W2f[:, kk, :],
                            8,
                        )
                        ysb = fin.tile([128, 512], F32, tag="ysb")
                        nc.scalar.copy(ysb[:], ps[:])
                        nc.sync.dma_start(
                            y_out.ap()[ts * 512 + mt * 128:ts * 512 + (mt + 1) * 128, :],
                            ysb[:],
                        )

    nc.compile()
    return nc


def kernel(**inputs):
    x = np.ascontiguousarray(np.asarray(inputs["x"], dtype=np.float32))
    w_q = np.asarray(inputs["w_q"], dtype=np.float32)
    w_k = np.asarray(inputs["w_k"], dtype=np.float32)
    w_v = np.asarray(inputs["w_v"], dtype=np.float32)
    W1 = np.asarray(inputs["mem_w1"], dtype=np.float32)
    W2 = np.asarray(inputs["mem_w2"], dtype=np.float32)

    def sig(t):
        return float(1.0 / (1.0 + np.exp(-np.float64(np.asarray(t)))))

    alpha = sig(inputs["alpha_t"])
    lr = sig(inputs["lr_t"])
    decay = sig(inputs["decay_t"])
    update_mem = int(np.asarray(inputs["update_mem"]))

    nc = build_program(update_mem, alpha, lr, decay)

    xT = np.ascontiguousarray(x.transpose(2, 0, 1))  # [512, B, S]
    xT4 = xT.reshape(4, 128, B, S)
    ident = np.eye(128, dtype=np.float32)
    wq_h = np.ascontiguousarray(w_q.reshape(4, 128, 512))
    wk_h = np.ascontiguousarray(w_k.reshape(4, 128, 512))
    wv_h = np.ascontiguousarray(w_v.reshape(4, 128, 512))

    in_maps = []
    for c in range(NCORES):
        xs = xT4[:, :, :, c * CPC * CH:(c + 1) * CPC * CH]  # [4,128,B,CPC*CH]
        xTp = np.ascontiguousarray(
            np.stack([xs[:, :, :, i * CH:(i + 1) * CH].reshape(4, 128, B * CH) for i in range(CPC)])
        )
        b, half = c // 2, c % 2
        xTq = np.ascontiguousarray(
            xT4[:, :, b, half * TOKQ:(half + 1) * TOKQ]
        )
        w1s = np.ascontiguousarray(
            W1[:, c * HS:(c + 1) * HS].reshape(4, 128, HS)
        )
        w2s = np.ascontiguousarray(W2[c * HS:(c + 1) * HS, :])
        w2ts = np.ascontiguousarray(w2s.T.reshape(4, 128, HS))
        in_maps.append({
            "xTp": xTp, "xTq": xTq,
            "wq": wq_h, "wk": wk_h, "wv": wv_h,
            "w1s": w1s, "w2s": w2s, "w2ts": w2ts,
            "ident": ident, "identr": ident,
        })

    res = run_bass_kernel_spmd(nc, in_maps, core_ids=list(range(NCORES)))

    y = np.empty((B, S, D), dtype=np.float32)
    for c in range(NCORES):
        b, half = c // 2, c % 2
        y[b, half * TOKQ:(half + 1) * TOKQ, :] = res.results[c]["y"]
    return y


# revision 9
# speedup vs baseline: 257.4050x; 257.4050x over previous
"""Trainium2 Bass kernel for nn_NeuralMemory (Titans-style chunked neural memory).

Strategy (8 NeuronCores, SPMD single program, shard identity carried by data):
  * Tensor-parallel over the memory MLP hidden dim (1024 -> 128 per core).
  * Prep: each core computes k/k^T/v/v^T for 2 of the 16 chunks; one AllGather
    shares them (replicated in HBM).
  * Scan (16 sequential chunk steps): forward h_s/a_s and pred-partial are
    hid-sharded; pred partials are AllReduced in bf16 (1 collective/step).
    Backward (g1_s, g2_s) and weight updates are local to each shard.
  * Final pass: tokens sharded 8-way; full W1/W2 assembled via one AllGather.
All matmuls run as float32r (tf32-like) with fp32 PSUM accumulation.
"""

import sys

sys.path.insert(0, "/opt/trn_rl_repo")

import numpy as np

import concourse.bass as bass  # noqa: F401
import concourse.tile as tile
from concourse import bacc, mybir
from concourse.bass_utils import run_bass_kernel_spmd

F32 = mybir.dt.float32
F32R = mybir.dt.float32r
BF16 = mybir.dt.bfloat16
AF = mybir.ActivationFunctionType
ALU = mybir.AluOpType

B, S, D = 4, 4096, 512
CH = 256
NCH = 16
TOK = B * CH  # 1024 tokens per chunk
H = 2 * D  # 1024
NCORES = 8
HS = H // NCORES  # 128 hid per core
CPC = NCH // NCORES  # chunks per core in prep = 2
TOKQ = (B * S) // NCORES  # 2048 final tokens per core
C2N = 2.0 / float(B * CH * D)
CHUNK_F = 2 * 512 * 1024  # floats per chunk in f32 AG buffer (kT + k)
CHUNK_H = 2 * 512 * 1024  # elems per chunk in bf16 AG buffer (v + vT)


def _mm_acc(nc, ps, lhsT_fn, rhs_fn, nk):
    for kk in range(nk):
        nc.tensor.matmul(ps, lhsT_fn(kk), rhs_fn(kk), start=(kk == 0), stop=(kk == nk - 1))


def build_program(update_mem, alpha, lr, decay, n_chunks=NCH):
    nc = bacc.Bacc("TRN2", target_bir_lowering=False, debug=False, num_devices=NCORES)

    # ---------------- I/O ----------------
    xTp_in = nc.dram_tensor("xTp", [CPC, 4, 128, B * CH], F32R, kind="ExternalInput")
    xTq_in = nc.dram_tensor("xTq", [4, 128, TOKQ], F32R, kind="ExternalInput")
    wq_in = nc.dram_tensor("wq", [4, 128, 512], F32R, kind="ExternalInput")
    wk_in = nc.dram_tensor("wk", [4, 128, 512], F32R, kind="ExternalInput")
    wv_in = nc.dram_tensor("wv", [4, 128, 512], F32R, kind="ExternalInput")
    w1s_in = nc.dram_tensor("w1s", [4, 128, HS], F32, kind="ExternalInput")
    w2s_in = nc.dram_tensor("w2s", [HS, 512], F32, kind="ExternalInput")
    w2ts_in = nc.dram_tensor("w2ts", [4, 128, HS], F32, kind="ExternalInput")
    ident_in = nc.dram_tensor("ident", [128, 128], F32, kind="ExternalInput")
    identr_in = nc.dram_tensor("identr", [128, 128], F32R, kind="ExternalInput")
    y_out = nc.dram_tensor("y", [TOKQ, 512], F32, kind="ExternalOutput")

    glr = -lr * C2N  # gradient copy-out scale (folds loss normalization)

    with tile.TileContext(nc) as tc:
        from contextlib import ExitStack

        with ExitStack() as ctx:
            const = ctx.enter_context(tc.tile_pool(name="const", bufs=1))
            psum = ctx.enter_context(tc.tile_pool(name="psum", bufs=4, space="PSUM"))
            dram = ctx.enter_context(tc.tile_pool(name="dram", bufs=1, space="DRAM"))
            dram2 = ctx.enter_context(tc.tile_pool(name="dram2", bufs=2, space="DRAM"))

            ident = const.tile([128, 128], F32, tag="ident")
            nc.sync.dma_start(ident[:], ident_in[:])
            identr_t = const.tile([128, 128], F32R, tag="identr")
            nc.sync.dma_start(identr_t[:], identr_in.ap())
            identr = identr_t[:]

            # persistent weight state (masters f32, matmul copies f32r)
            W1m = const.tile([128, 4, HS], F32, tag="W1m")
            W2m = const.tile([128, 512], F32, tag="W2m")
            W2Tm = const.tile([128, 4, HS], F32, tag="W2Tm")
            nc.sync.dma_start(W1m[:], w1s_in.ap().rearrange("k p m -> p k m"))
            nc.sync.dma_start(W2m[:], w2s_in[:])
            nc.sync.dma_start(W2Tm[:], w2ts_in.ap().rearrange("k p m -> p k m"))
            W1r = const.tile([128, 4, HS], F32R, tag="W1r")
            W2r = const.tile([128, 512], F32R, tag="W2r")
            W2Tr = const.tile([128, 4, HS], F32R, tag="W2Tr")
            nc.vector.tensor_copy(W1r[:], W1m[:])
            nc.vector.tensor_copy(W2r[:], W2m[:])
            nc.vector.tensor_copy(W2Tr[:], W2Tm[:])
            S1 = const.tile([128, 4, HS], F32, tag="S1")
            S2 = const.tile([128, 512], F32, tag="S2")
            S2T = const.tile([128, 4, HS], F32, tag="S2T")
            nc.vector.memset(S1[:], 0.0)
            nc.vector.memset(S2[:], 0.0)
            nc.vector.memset(S2T[:], 0.0)

            # DRAM scratch
            qTd = dram.tile([4, 128, TOKQ], F32R, tag="qTd")
            agin_f = dram.tile([CPC, CHUNK_F], F32, tag="agin_f")
            agout_f = dram.tile([NCORES, CPC, CHUNK_F], F32, tag="agout_f", addr_space="Shared")
            agin_h = dram.tile([CPC, CHUNK_H], BF16, tag="agin_h")
            agout_h = dram.tile([NCORES, CPC, CHUNK_H], BF16, tag="agout_h", addr_space="Shared")
            wag_in = dram.tile([4 * 128 * HS + HS * 512], F32, tag="wag_in")
            wag_out = dram.tile([NCORES, 4 * 128 * HS + HS * 512], F32, tag="wag_out", addr_space="Shared")

            # ---------------- prep phase ----------------
            with tc.tile_pool(name="prep", bufs=2) as prep:
                wq = prep.tile([128, 4, 512], F32R, tag="wq")
                wk = prep.tile([128, 4, 512], F32R, tag="wk")
                wv = prep.tile([128, 4, 512], F32R, tag="wv")
                nc.sync.dma_start(wq[:], wq_in.ap().rearrange("k p n -> p k n"))
                nc.sync.dma_start(wk[:], wk_in.ap().rearrange("k p n -> p k n"))
                nc.sync.dma_start(wv[:], wv_in.ap().rearrange("k p n -> p k n"))

                # qT for the final pass (token shard of this core)
                xTq = prep.tile([128, 4, TOKQ], F32R, tag="xTq")
                nc.sync.dma_start(xTq[:], xTq_in.ap().rearrange("k p n -> p k n"))
                for m in range(4):
                    for ns in range(TOKQ // 512):
                        ps = psum.tile([128, 512], F32, tag="mm")
                        _mm_acc(
                            nc, ps[:],
                            lambda kk, m=m: wq[:, kk, m * 128:(m + 1) * 128],
                            lambda kk, ns=ns: xTq[:, kk, ns * 512:(ns + 1) * 512],
                            4,
                        )
                        qs = prep.tile([128, 512], F32R, tag="qstage")
                        nc.vector.tensor_copy(qs[:], ps[:])
                        nc.sync.dma_start(
                            qTd[:, :, ns * 512:(ns + 1) * 512]
                            .rearrange("k p n -> p k n")[:, m, :],
                            qs[:],
                        )

                if update_mem:
                    for i in range(CPC):
                        XTp = prep.tile([128, 4, B * CH], F32R, tag="XTp")
                        nc.sync.dma_start(
                            XTp[:],
                            xTp_in.ap()[i].rearrange("k p n -> p k n"),
                        )
                        XTf = XTp

                        # kT chunk: [512(d2), 1024(tok)] stored [m p n]
                        for m in range(4):
                            for nh in range(2):
                                ps = psum.tile([128, 512], F32, tag="mm")
                                _mm_acc(
                                    nc, ps[:],
                                    lambda kk, m=m: wk[:, kk, m * 128:(m + 1) * 128],
                                    lambda kk, nh=nh: XTf[:, kk, nh * 512:(nh + 1) * 512],
                                    4,
                                )
                                st = prep.tile([128, 512], F32, tag="pstage")
                                nc.scalar.copy(st[:], ps[:])
                                nc.sync.dma_start(
                                    agin_f[i, 0:512 * 1024]
                                    .rearrange("(m p n) -> p m n", m=4, p=128, n=1024)
                                    [:, m, nh * 512:(nh + 1) * 512],
                                    st[:],
                                )
                        # k chunk: [1024(tok), 512] stored [mt p n]
                        for mt in range(8):
                            ps = psum.tile([128, 512], F32, tag="mm")
                            _mm_acc(
                                nc, ps[:],
                                lambda kk, mt=mt: XTp[:, kk, mt * 128:(mt + 1) * 128],
                                lambda kk: wk[:, kk, :],
                                4,
                            )
                            st = prep.tile([128, 512], F32, tag="pstage")
                            nc.scalar.copy(st[:], ps[:])
                            nc.sync.dma_start(
                                agin_f[i, 512 * 1024:2 * 512 * 1024]
                                .rearrange("(mt p n) -> p mt n", mt=8, p=128, n=512)[:, mt, :],
                                st[:],
                            )
                        # v chunk (bf16): [1024(tok), 512] + vT [512(d2), 1024]
                        for mt in range(8):
                            ps = psum.tile([128, 512], F32, tag="mm")
                            _mm_acc(
                                nc, ps[:],
                                lambda kk, mt=mt: XTp[:, kk, mt * 128:(mt + 1) * 128],
                                lambda kk: wv[:, kk, :],
                                4,
                            )
                            sth = prep.tile([128, 512], BF16, tag="hstage")
                            nc.vector.tensor_copy(sth[:], ps[:])
                            nc.sync.dma_start(
                                agin_h[i, 0:512 * 1024]
                                .rearrange("(mt p n) -> p mt n", mt=8, p=128, n=512)[:, mt, :],
                                sth[:],
                            )
                        for m in range(4):
                            for nh in range(2):
                                ps = psum.tile([128, 512], F32, tag="mm")
                                _mm_acc(
                                    nc, ps[:],
                                    lambda kk, m=m: wv[:, kk, m * 128:(m + 1) * 128],
                                    lambda kk, nh=nh: XTf[:, kk, nh * 512:(nh + 1) * 512],
                                    4,
                                )
                                sth = prep.tile([128, 512], BF16, tag="hstage")
                                nc.vector.tensor_copy(sth[:], ps[:])
                                nc.sync.dma_start(
                                    agin_h[i, 512 * 1024:2 * 512 * 1024]
                                    .rearrange("(m p n) -> p m n", m=4, p=128, n=1024)
                                    [:, m, nh * 512:(nh + 1) * 512],
                                    sth[:],
                                )

            if update_mem:
                nc.gpsimd.collective_compute(
                    "AllGather",
                    ALU.bypass,
                    replica_groups=[list(range(NCORES))],
                    ins=[agin_f[:].opt()],
                    outs=[agout_f[:].opt()],
                )
                nc.gpsimd.collective_compute(
                    "AllGather",
                    ALU.bypass,
                    replica_groups=[list(range(NCORES))],
                    ins=[agin_h[:].opt()],
                    outs=[agout_h[:].opt()],
                )

            # ---------------- scan ----------------
            if update_mem:
                with tc.tile_pool(name="scan", bufs=1) as sc:
                    for t in range(n_chunks):
                        r, i = t // CPC, t % CPC
                        kT_t = sc.tile([128, 4, 1024], F32R, tag="kT_t", bufs=1)
                        nc.sync.dma_start(
                            kT_t[:],
                            agout_f[r, i, 0:512 * 1024]
                            .rearrange("(m p n) -> p m n", m=4, p=128, n=1024)
                            .bitcast(F32R),
                        )
                        k_t = sc.tile([128, 8, 512], F32R, tag="k_t", bufs=2)
                        nc.sync.dma_start(
                            k_t[:],
                            agout_f[r, i, 512 * 1024:2 * 512 * 1024]
                            .rearrange("(mt p n) -> p mt n", mt=8, p=128, n=512)
                            .bitcast(F32R),
                        )
                        v_t = sc.tile([128, 8, 512], BF16, tag="v_t", bufs=1)
                        nc.sync.dma_start(
                            v_t[:],
                            agout_h[r, i, 0:512 * 1024]
                            .rearrange("(mt p n) -> p mt n", mt=8, p=128, n=512),
                        )
                        vT_t = sc.tile([128, 4, 1024], BF16, tag="vT_t", bufs=1)
                        nc.sync.dma_start(
                            vT_t[:],
                            agout_h[r, i, 512 * 1024:2 * 512 * 1024]
                            .rearrange("(m p n) -> p m n", m=4, p=128, n=1024),
                        )

                        # forward: h_s^T, a_s^T  [128(hid_s), 1024(tok)]
                        h_sT = sc.tile([128, 1024], F32, tag="h_sT", bufs=2)
                        a_sT = sc.tile([128, 1024], F32R, tag="a_sT", bufs=2)
                        for nh in range(2):
                            ps = psum.tile([128, 512], F32, tag="mm")
                            _mm_acc(
                                nc, ps[:],
                                lambda kk: W1r[:, kk, :],
                                lambda kk, nh=nh: kT_t[:, kk, nh * 512:(nh + 1) * 512],
                                4,
                            )
                            nc.scalar.copy(h_sT[:, nh * 512:(nh + 1) * 512], ps[:])
                            nc.scalar.activation(
                                a_sT[:, nh * 512:(nh + 1) * 512], ps[:], AF.Gelu
                            )

                        # a_s token layout via PE transpose
                        a_s = sc.tile([128, 8, 128], F32R, tag="a_s", bufs=2)
                        for j in range(8):
                            tp = psum.tile([128, 128], F32, tag="tp")
                            nc.tensor.transpose(tp[:].bitcast(F32R), a_sT[:, j * 128:(j + 1) * 128], identr)
                            nc.vector.tensor_copy(a_s[:, j, :], tp[:])

                        # pred partial (token layout) -> bf16 -> AllReduce
                        arin = dram2.tile([TOK * 512], BF16, tag="arin")
                        for mt in range(8):
                            ps = psum.tile([128, 512], F32, tag="mm")
                            nc.tensor.matmul(
                                ps[:], a_sT[:, mt * 128:(mt + 1) * 128], W2r[:],
                                start=True, stop=True,
                            )
                            pb = sc.tile([128, 512], BF16, tag="partb", bufs=2)
                            nc.vector.tensor_copy(pb[:], ps[:])
                            nc.sync.dma_start(
                                arin[mt * 65536:(mt + 1) * 65536]
                                .rearrange("(p n) -> p n", p=128, n=512),
                                pb[:],
                            )
                        arout = dram2.tile([TOK * 512], BF16, tag="arout", addr_space="Shared")
                        nc.gpsimd.collective_compute(
                            "AllReduce",
                            ALU.add,
                            replica_groups=[list(range(NCORES))],
                            ins=[arin[:].opt()],
                            outs=[arout[:].opt()],
                        )

                        pred = sc.tile([128, 8, 512], BF16, tag="pred")
                        nc.sync.dma_start(
                            pred[:],
                            arout[:].rearrange("(mt p n) -> p mt n", mt=8, p=128, n=512),
                        )
                        predT = sc.tile([128, 4, 1024], BF16, tag="predT")
                        ar2d = arout[:].rearrange("(t n) -> t n", t=1024, n=512)
                        for kk in range(4):
                            nc.sync.dma_start_transpose(
                                predT[:, kk, :], ar2d[:, kk * 128:(kk + 1) * 128]
                            )

                        # d_pred (token, f32r) and d_pred^T (feature, f32r); raw (pred - v)
                        d_pred = sc.tile([128, 8, 512], F32R, tag="d_pred")
                        nc.vector.tensor_sub(
                            d_pred[:].rearrange("p a b -> p (a b)"),
                            pred[:].rearrange("p a b -> p (a b)"),
                            v_t[:].rearrange("p a b -> p (a b)"),
                        )
                        d_predT = sc.tile([128, 4, 1024], F32R, tag="d_predT")
                        nc.vector.tensor_sub(
                            d_predT[:].rearrange("p a b -> p (a b)"),
                            predT[:].rearrange("p a b -> p (a b)"),
                            vT_t[:].rearrange("p a b -> p (a b)"),
                        )

                        # g2_s = a_s^T d_pred (hid_s x 512); scaled by -lr*c2N on copy-out
                        g2l = sc.tile([128, 512], F32, tag="g2l")
                        ps = psum.tile([128, 512], F32, tag="mm")
                        _mm_acc(
                            nc, ps[:],
                            lambda kk: a_s[:, kk, :],
                            lambda kk: d_pred[:, kk, :],
                            8,
                        )
                        nc.scalar.mul(g2l[:], ps[:], glr)
                        # g2T_s (512 x hid_s) stored [128,4,128]
                        g2Tl = sc.tile([128, 4, 128], F32, tag="g2Tl")
                        for m in range(4):
                            tp = psum.tile([128, 128], F32, tag="tp")
                            _mm_acc(
                                nc, tp[:],
                                lambda kk, m=m: d_pred[:, kk, m * 128:(m + 1) * 128],
                                lambda kk: a_s[:, kk, :],
                                8,
                            )
                            nc.scalar.mul(g2Tl[:, m, :], tp[:], glr)

                        # d_a^T = W2_s @ d_pred^T ; d_h^T = d_a^T * gelu'(h^T)
                        d_hT = sc.tile([128, 1024], F32R, tag="d_hT")
                        for nh in range(2):
                            ps = psum.tile([128, 512], F32, tag="mm")
                            _mm_acc(
                                nc, ps[:],
                                lambda kk: W2Tr[:, kk, :],
                                lambda kk, nh=nh: d_predT[:, kk, nh * 512:(nh + 1) * 512],
                                4,
                            )
                            dg = sc.tile([128, 512], F32, tag="dg", bufs=2)
                            nc.scalar.activation(
                                dg[:], h_sT[:, nh * 512:(nh + 1) * 512], AF.Derivative_Gelu
                            )
                            nc.vector.tensor_mul(
                                d_hT[:, nh * 512:(nh + 1) * 512], dg[:], ps[:]
                            )

                        # d_h token layout
                        d_h = sc.tile([128, 8, 128], F32R, tag="d_h")
                        for j in range(8):
                            tp = psum.tile([128, 128], F32, tag="tp")
                            nc.tensor.transpose(tp[:].bitcast(F32R), d_hT[:, j * 128:(j + 1) * 128], identr)
                            nc.vector.tensor_copy(d_h[:, j, :], tp[:])

                        # g1T_s [hid_s, 512] then transpose to g1l [512, hid_s]
                        g1Tl = sc.tile([128, 512], F32, tag="g1Tl")
                        ps = psum.tile([128, 512], F32, tag="mm")
                        _mm_acc(
                            nc, ps[:],
                            lambda kk: d_h[:, kk, :],
                            lambda kk: k_t[:, kk, :],
                            8,
                        )
                        nc.scalar.mul(g1Tl[:], ps[:], glr)
                        g1l = sc.tile([128, 4, 128], F32, tag="g1l")
                        for m in range(4):
                            tp = psum.tile([128, 128], F32, tag="tp")
                            nc.tensor.transpose(tp[:], g1Tl[:, m * 128:(m + 1) * 128], ident[:])
                            nc.vector.tensor_copy(g1l[:, m, :], tp[:])

                        # updates: S = decay*S + gl ; W = (1-alpha)*W + S ; refresh f32r copies
                        nc.vector.scalar_tensor_tensor(
                            S1[:].rearrange("p a b -> p (a b)"), S1[:].rearrange("p a b -> p (a b)"),
                            decay, g1l[:].rearrange("p a b -> p (a b)"), ALU.mult, ALU.add,
                        )
                        nc.vector.scalar_tensor_tensor(
                            W1m[:].rearrange("p a b -> p (a b)"), W1m[:].rearrange("p a b -> p (a b)"),
                            1.0 - alpha, S1[:].rearrange("p a b -> p (a b)"), ALU.mult, ALU.add,
                        )
                        nc.vector.tensor_copy(W1r[:], W1m[:])
                        nc.vector.scalar_tensor_tensor(
                            S2[:], S2[:], decay, g2l[:], ALU.mult, ALU.add
                        )
                        nc.vector.scalar_tensor_tensor(
                            W2m[:], W2m[:], 1.0 - alpha, S2[:], ALU.mult, ALU.add
                        )
                        nc.vector.tensor_copy(W2r[:], W2m[:])
                        nc.vector.scalar_tensor_tensor(
                            S2T[:].rearrange("p a b -> p (a b)"), S2T[:].rearrange("p a b -> p (a b)"),
                            decay, g2Tl[:].rearrange("p a b -> p (a b)"), ALU.mult, ALU.add,
                        )
                        nc.vector.scalar_tensor_tensor(
                            W2Tm[:].rearrange("p a b -> p (a b)"), W2Tm[:].rearrange("p a b -> p (a b)"),
                            1.0 - alpha, S2T[:].rearrange("p a b -> p (a b)"), ALU.mult, ALU.add,
                        )
                        nc.vector.tensor_copy(W2Tr[:], W2Tm[:])

            # ---------------- gather full W, final pass ----------------
            nc.sync.dma_start(
                wag_in[0:4 * 128 * HS].rearrange("(k p m) -> p k m", k=4, p=128, m=HS),
                W1m[:],
            )
            nc.sync.dma_start(
                wag_in[4 * 128 * HS:].rearrange("(p n) -> p n", p=HS, n=512),
                W2m[:],
            )
            nc.gpsimd.collective_compute(
                "AllGather",
                ALU.bypass,
                replica_groups=[list(range(NCORES))],
                ins=[wag_in[:].opt()],
                outs=[wag_out[:].opt()],
            )

            with tc.tile_pool(name="fin", bufs=2) as fin:
                W1f = fin.tile([128, 4, 8, 128], F32R, tag="W1f", bufs=1)
                for kk in range(4):
                    nc.sync.dma_start(
                        W1f[:, kk, :, :],
                        wag_out[:, kk * 128 * HS:(kk + 1) * 128 * HS]
                        .rearrange("r (p m) -> p r m", p=128, m=HS)
                        .bitcast(F32R),
                    )
                W2f = fin.tile([128, 8, 512], F32R, tag="W2f", bufs=1)
                nc.sync.dma_start(
                    W2f[:],
                    wag_out[:, 4 * 128 * HS:]
                    .rearrange("r (p n) -> p r n", p=HS, n=512)
                    .bitcast(F32R),
                )

                for ts in range(TOKQ // 512):
                    qT_sl = fin.tile([128, 4, 512], F32R, tag="qT_sl")
                    nc.sync.dma_start(
                        qT_sl[:],
                        qTd[:, :, ts * 512:(ts + 1) * 512].rearrange("k p n -> p k n"),
                    )
                    aT_sl = fin.tile([128, 8, 512], F32R, tag="aT_sl")
                    for rr in range(8):
                        ps = psum.tile([128, 512], F32, tag="mm")
                        _mm_acc(
                            nc, ps[:],
                            lambda kk, rr=rr: W1f[:, kk, rr, :],
                            lambda kk: qT_sl[:, kk, :],
                            4,
                        )
                        nc.scalar.activation(aT_sl[:, rr, :], ps[:], AF.Gelu)
                    for mt in range(4):
                        ps = psum.tile([128, 512], F32, tag="mm")
                        _mm_acc(
                            nc, ps[:],
                            lambda kk, mt=mt: aT_sl[:, kk, mt * 128:(mt + 1) * 128],
                            lambda kk: W2f[:, kk, :],
                            8,
                        )
                        ysb = fin.tile([128, 512], F32, tag="ysb")
                        nc.scalar.copy(ysb[:], ps[:])
                        nc.sync.dma_start(
                            y_out.ap()[ts * 512 + mt * 128:ts * 512 + (mt + 1) * 128, :],
                            ysb[:],
                        )

    nc.compile()
    return nc


def prepare(inputs):
    """Build the Bass program and per-core input maps."""
    x = np.ascontiguousarray(np.asarray(inputs["x"], dtype=np.float32))
    w_q = np.asarray(inputs["w_q"], dtype=np.float32)
    w_k = np.asarray(inputs["w_k"], dtype=np.float32)
    w_v = np.asarray(inputs["w_v"], dtype=np.float32)
    W1 = np.asarray(inputs["mem_w1"], dtype=np.float32)
    W2 = np.asarray(inputs["mem_w2"], dtype=np.float32)

    def sig(t):
        return float(1.0 / (1.0 + np.exp(-np.float64(np.asarray(t)))))

    alpha = sig(inputs["alpha_t"])
    lr = sig(inputs["lr_t"])
    decay = sig(inputs["decay_t"])
    update_mem = int(np.asarray(inputs["update_mem"]))

    nc = build_program(update_mem, alpha, lr, decay)

    xT = np.ascontiguousarray(x.transpose(2, 0, 1))  # [512, B, S]
    xT4 = xT.reshape(4, 128, B, S)
    ident = np.eye(128, dtype=np.float32)
    wq_h = np.ascontiguousarray(w_q.reshape(4, 128, 512))
    wk_h = np.ascontiguousarray(w_k.reshape(4, 128, 512))
    wv_h = np.ascontiguousarray(w_v.reshape(4, 128, 512))

    in_maps = []
    for c in range(NCORES):
        xs = xT4[:, :, :, c * CPC * CH:(c + 1) * CPC * CH]  # [4,128,B,CPC*CH]
        xTp = np.ascontiguousarray(
            np.stack([xs[:, :, :, i * CH:(i + 1) * CH].reshape(4, 128, B * CH) for i in range(CPC)])
        )
        b, half = c // 2, c % 2
        xTq = np.ascontiguousarray(
            xT4[:, :, b, half * TOKQ:(half + 1) * TOKQ]
        )
        w1s = np.ascontiguousarray(
            W1[:, c * HS:(c + 1) * HS].reshape(4, 128, HS)
        )
        w2s = np.ascontiguousarray(W2[c * HS:(c + 1) * HS, :])
        w2ts = np.ascontiguousarray(w2s.T.reshape(4, 128, HS))
        in_maps.append({
            "xTp": xTp, "xTq": xTq,
            "wq": wq_h, "wk": wk_h, "wv": wv_h,
            "w1s": w1s, "w2s": w2s, "w2ts": w2ts,
            "ident": ident, "identr": ident,
        })

    return nc, in_maps


def kernel(**inputs):
    nc, in_maps = prepare(inputs)
    res = run_bass_kernel_spmd(nc, in_maps, core_ids=list(range(NCORES)))

    y = np.empty((B, S, D), dtype=np.float32)
    for c in range(NCORES):
        b, half = c // 2, c % 2
        y[b, half * TOKQ:(half + 1) * TOKQ, :] = res.results[c]["y"]
    return y
